# revision 10
# baseline (speedup 1.0000x reference)
"""Multi-head attention (b=4, s=2048, E=1024, 16 heads, d_k=64) on 8 trn2 cores.

Sharding: core = (batch b in 0..3, head-group g in 0..1); each core handles one
batch and 8 of the 16 heads (Megatron-style column-parallel QKV + row-parallel
out-proj). Each core returns its partial out-projection [2048, 1024]; the host
sums each batch's pair of partials in fp32 (the 2-way all-reduce, done during
unsharding).

All matmuls run in float32r (TF32-like, full PE rate). Scores are computed
transposed ([k, q] layout, 2 heads row-packed in the PE array), exp on the
scalar engine straight out of PSUM with the 1/sqrt(64) scale fused, attn@V
col-packed 2 heads, softmax denominator via DVE accumulation + ones-matmul
partition reduce/broadcast + DVE reciprocal applied on PSUM eviction.
"""

import sys

if "/opt/trn_rl_repo" not in sys.path:
    sys.path.insert(0, "/opt/trn_rl_repo")

import numpy as np

N_CORES = 8
B, S, E, H, DK = 4, 2048, 1024, 16, 64
GH = 512          # head-group width: 8 heads * 64
QB = 512          # query block (free dim of scores matmuls)
N_QB = S // QB    # 4
N_KT = S // 128   # 16 k-tiles
N_EC = E // 128   # 8 contraction chunks for projections
N_M = GH // 128   # 4 dq tiles / head pairs

_NC = None


def build_nc():
    import concourse.tile as tile
    from concourse import bacc, mybir

    f32 = mybir.dt.float32
    f32r = mybir.dt.float32r
    Exp = mybir.ActivationFunctionType.Exp

    nc = bacc.Bacc("TRN2", target_bir_lowering=False, debug=False,
                   num_devices=N_CORES)

    xT = nc.dram_tensor("xT", [E, S], f32r, kind="ExternalInput")
    wq = nc.dram_tensor("wq", [E, GH], f32r, kind="ExternalInput")
    wk = nc.dram_tensor("wk", [E, GH], f32r, kind="ExternalInput")
    wv = nc.dram_tensor("wv", [E, GH], f32r, kind="ExternalInput")
    wo = nc.dram_tensor("wo", [GH, E], f32r, kind="ExternalInput")
    y = nc.dram_tensor("y", [S, E], f32, kind="ExternalOutput")

    with tile.TileContext(nc) as tc:
        with tc.tile_pool(name="persist", bufs=1) as persist, \
             tc.tile_pool(name="ps_mm", bufs=2, space="PSUM") as ps_mm, \
             tc.tile_pool(name="ps_sa", bufs=2, space="PSUM") as ps_sa, \
             tc.tile_pool(name="ps_sb", bufs=2, space="PSUM") as ps_sb, \
             tc.tile_pool(name="ps_oa", bufs=1, space="PSUM") as ps_oa, \
             tc.tile_pool(name="ps_ob", bufs=1, space="PSUM") as ps_ob:

            QT = persist.tile([128, N_M, S], f32r)   # [p, m, s]: q^T rows m*128+p
            KT = persist.tile([128, N_M, S], f32r)
            V = persist.tile([128, N_KT, GH], f32r)  # [p, st, dv]: V rows st*128+p
            ones = persist.tile([128, 128], f32r)
            ones_f = persist.tile([128, 128], f32)
            nc.vector.memset(ones_f[:], 1.0)
            nc.vector.tensor_copy(ones[:], ones_f[:])

            # ---------- phase 1: projections ----------
            with tc.tile_pool(name="xt", bufs=1) as xt_pool, \
                 tc.tile_pool(name="wchunk", bufs=10) as wch, \
                 tc.tile_pool(name="wvchunk", bufs=8) as wvch:
                xt = xt_pool.tile([128, N_EC, S], f32r)
                for ec in range(N_EC):
                    nc.sync.dma_start(xt[:, ec, :], xT[ec * 128:(ec + 1) * 128, :])

                # Q^T and K^T: [dq, s] tiles; lhsT = w chunk, rhs = x^T chunk
                for dst, w_dram, wname in ((QT, wq, "q"), (KT, wk, "k")):
                    for m in range(N_M):
                        chunks = []
                        for ec in range(N_EC):
                            c = wch.tile([128, 128], f32r, name=f"w{wname}c")
                            nc.sync.dma_start(
                                c[:], w_dram[ec * 128:(ec + 1) * 128,
                                             m * 128:(m + 1) * 128])
                            chunks.append(c)
                        for sb in range(N_QB):
                            ps = ps_mm.tile([128, QB], f32, name="mm")
                            for ec in range(N_EC):
                                nc.tensor.matmul(
                                    ps[:], chunks[ec][:],
                                    xt[:, ec, sb * QB:(sb + 1) * QB],
                                    start=(ec == 0), stop=(ec == N_EC - 1))
                            nc.vector.tensor_copy(
                                dst[:, m, sb * QB:(sb + 1) * QB], ps[:])

                # V: [s, dv] tiles; lhsT = x^T chunk, rhs = w chunk
                vchunks = []
                for ec in range(N_EC):
                    c = wvch.tile([128, GH], f32r, name="wvc")
                    nc.sync.dma_start(c[:], wv[ec * 128:(ec + 1) * 128, :])
                    vchunks.append(c)
                for st in range(N_KT):
                    ps = ps_mm.tile([128, GH], f32, name="mm")
                    for ec in range(N_EC):
                        nc.tensor.matmul(
                            ps[:], xt[:, ec, st * 128:(st + 1) * 128],
                            vchunks[ec][:],
                            start=(ec == 0), stop=(ec == N_EC - 1))
                    nc.vector.tensor_copy(V[:, st, :], ps[:])

            # ---------- phase 2: attention + out-proj, per query block ----------
            with tc.tile_pool(name="wop", bufs=1) as wop, \
                 tc.tile_pool(name="expp", bufs=6) as expp, \
                 tc.tile_pool(name="accp", bufs=2) as accp, \
                 tc.tile_pool(name="recipp", bufs=2) as recipp, \
                 tc.tile_pool(name="outTp", bufs=2) as outTp, \
                 tc.tile_pool(name="finp", bufs=4) as finp:
                wo_sb = wop.tile([128, N_M, E], f32r)
                for fo in range(N_M):
                    nc.sync.dma_start(wo_sb[:, fo, :], wo[fo * 128:(fo + 1) * 128, :])
                for qb in range(N_QB):
                    qsl = slice(qb * QB, (qb + 1) * QB)
                    outT = outTp.tile([128, N_M, QB], f32r)  # [f, j, q] for this qb
                    for j in range(N_M):  # head pair j = heads (2j, 2j+1)
                        oTA = ps_oa.tile([64, QB], f32, name="ps_oTA")
                        oTB = ps_ob.tile([64, QB], f32, name="ps_oTB")
                        accA = accp.tile([128, QB], f32r, name="accA")
                        accB = accp.tile([128, QB], f32r, name="accB")
                        for kt in range(N_KT):
                            ksl = slice(kt * 128, (kt + 1) * 128)
                            # scores^T for the two heads, row-packed (d=64 each)
                            psA = ps_sa.tile([128, QB], f32, name="psA")
                            nc.tensor.matmul(psA[:], KT[0:64, j, ksl],
                                             QT[0:64, j, qsl])
                            psB = ps_sb.tile([128, QB], f32, name="psB")
                            nc.tensor.matmul(psB[:], KT[64:128, j, ksl],
                                             QT[64:128, j, qsl])
                            # exp(score/8): PSUM -> SBUF f32r
                            eA = expp.tile([128, QB], f32r, name="eA")
                            nc.scalar.activation(eA[:], psA[:], Exp, scale=0.125)
                            eB = expp.tile([128, QB], f32r, name="eB")
                            nc.scalar.activation(eB[:], psB[:], Exp, scale=0.125)
                            # attn^T @ V -> outT psum (M=64 per head)
                            nc.tensor.matmul(oTA[:, :],
                                             V[:, kt, (2 * j) * DK:(2 * j + 1) * DK],
                                             eA[:], start=(kt == 0),
                                             stop=(kt == N_KT - 1))
                            nc.tensor.matmul(oTB[:, :],
                                             V[:, kt, (2 * j + 1) * DK:(2 * j + 2) * DK],
                                             eB[:], start=(kt == 0),
                                             stop=(kt == N_KT - 1))
                            # denominator accumulation
                            if kt == 0:
                                nc.vector.tensor_copy(accA[:], eA[:])
                                nc.vector.tensor_copy(accB[:], eB[:])
                            else:
                                nc.vector.tensor_add(accA[:], accA[:], eA[:])
                                nc.vector.tensor_add(accB[:], accB[:], eB[:])
                        # partition-reduce + broadcast the denominators
                        bcA = ps_mm.tile([128, QB], f32, name="mm")
                        nc.tensor.matmul(bcA[:], ones[:], accA[:])
                        bcB = ps_mm.tile([128, QB], f32, name="mm")
                        nc.tensor.matmul(bcB[:], ones[:], accB[:])
                        rA = recipp.tile([128, QB], f32, name="rA")
                        nc.vector.reciprocal(rA[:], bcA[:])
                        rB = recipp.tile([128, QB], f32, name="rB")
                        nc.vector.reciprocal(rB[:], bcB[:])
                        # evict outT with the softmax divide fused
                        # (B write is partition-shifted: psum rows 0:64 ->
                        # sbuf rows 64:128)
                        nc.vector.tensor_mul(outT[0:64, j, :], oTA[:, :],
                                             rA[0:64, :])
                        nc.vector.tensor_mul(outT[64:128, j, :], oTB[:, :],
                                             rB[0:64, :])
                    # out-proj for this query block: y[s, e] += outT^T @ wo
                    for st in range(QB // 128):
                        row0 = qb * QB + st * 128
                        for eb in range(E // QB):
                            ps = ps_mm.tile([128, QB], f32, name="mm")
                            for jj in range(N_M):
                                nc.tensor.matmul(
                                    ps[:], outT[:, jj, st * 128:(st + 1) * 128],
                                    wo_sb[:, jj, eb * QB:(eb + 1) * QB],
                                    start=(jj == 0), stop=(jj == N_M - 1))
                            fo = finp.tile([128, QB], f32, name="fo")
                            nc.vector.tensor_copy(fo[:], ps[:])
                            nc.sync.dma_start(
                                y[row0:row0 + 128, eb * QB:(eb + 1) * QB], fo[:])

    nc.finalize()
    return nc


def get_nc():
    global _NC
    if _NC is None:
        _NC = build_nc()
    return _NC


def make_in_maps(x, Wq, Wk, Wv, Wo):
    in_maps = []
    for b in range(B):
        for g in range(2):
            sl = slice(g * GH, (g + 1) * GH)
            in_maps.append(dict(
                xT=np.ascontiguousarray(x[b].T),
                wq=np.ascontiguousarray(Wq[sl, :].T),
                wk=np.ascontiguousarray(Wk[sl, :].T),
                wv=np.ascontiguousarray(Wv[sl, :].T),
                wo=np.ascontiguousarray(Wo[:, sl].T),
            ))
    return in_maps


def combine(results):
    out = np.empty((B, S, E), np.float32)
    for b in range(B):
        out[b] = results[2 * b]["y"] + results[2 * b + 1]["y"]
    return out


def kernel(x, Wq, Wk, Wv, Wo):
    from concourse.bass_utils import run_bass_kernel_spmd
    x = np.asarray(x, np.float32)
    res = run_bass_kernel_spmd(
        get_nc(),
        make_in_maps(x, np.asarray(Wq, np.float32), np.asarray(Wk, np.float32),
                     np.asarray(Wv, np.float32), np.asarray(Wo, np.float32)),
        core_ids=list(range(N_CORES)))
    return combine(res.results)


# revision 17
# speedup vs baseline: 1.2272x; 1.2272x over previous
"""Multi-head attention (b=4, s=2048, E=1024, 16 heads, d_k=64) on 8 trn2 cores.

Sharding: core = (batch b in 0..3, head-group g in 0..1); each core handles one
batch and 8 of the 16 heads (Megatron-style column-parallel QKV + row-parallel
out-proj). Each core returns its partial out-projection [2048, 1024]; the host
sums each batch's pair of partials in fp32 (the 2-way all-reduce, done during
unsharding).

Matmul operands are bf16 (fp32 PSUM accumulation). Scores are computed
transposed ([k, q] layout, two heads row-packed in the PE array, both written
into one 2-bank PSUM tile so a single ACT instruction applies exp with the
1/sqrt(64) scale fused). V carries an extra all-ones column so the attn^T @ V
matmul also produces the softmax denominator for free; a tiny batched DVE
reciprocal + ones-matmul broadcasts 1/denom across partitions, applied on the
PSUM eviction, which feeds the out-projection directly.
"""

import sys

if "/opt/trn_rl_repo" not in sys.path:
    sys.path.insert(0, "/opt/trn_rl_repo")

import numpy as np
import ml_dtypes

N_CORES = 8
B, S, E, H, DK = 4, 2048, 1024, 16, 64
GH = 512          # head-group width: 8 heads * 64
QB = 512          # query block (free dim of scores matmuls)
N_QB = S // QB    # 4
N_KT = S // 128   # 16 k-tiles
N_EC = E // 128   # 8 contraction chunks for projections
N_M = GH // 128   # 4 dq tiles / head pairs
DKE = DK + 1      # V head width incl the ones column

_NC = None


def build_nc():
    import concourse.tile as tile
    from concourse import bacc, mybir

    f32 = mybir.dt.float32
    bf16 = mybir.dt.float16  # fp16: same PE rate, 4x better mantissa than bf16
    Exp = mybir.ActivationFunctionType.Exp

    nc = bacc.Bacc("TRN2", target_bir_lowering=False, debug=False,
                   num_devices=N_CORES)

    xT = nc.dram_tensor("xT", [E, S], bf16, kind="ExternalInput")
    wq = nc.dram_tensor("wq", [E, GH], bf16, kind="ExternalInput")
    wk = nc.dram_tensor("wk", [E, GH], bf16, kind="ExternalInput")
    wv = nc.dram_tensor("wv", [E, GH], bf16, kind="ExternalInput")
    wo = nc.dram_tensor("wo", [GH, E], bf16, kind="ExternalInput")
    sel = nc.dram_tensor("sel", [2, 128], f32, kind="ExternalInput")
    y = nc.dram_tensor("y", [S, E], f32, kind="ExternalOutput")

    with tile.TileContext(nc) as tc:
        with tc.tile_pool(name="persist", bufs=1) as persist, \
             tc.tile_pool(name="ps_mm", bufs=2, space="PSUM") as ps_mm, \
             tc.tile_pool(name="ps_sc", bufs=2, space="PSUM") as ps_sc, \
             tc.tile_pool(name="ps_oa", bufs=1, space="PSUM") as ps_oa, \
             tc.tile_pool(name="ps_ob", bufs=1, space="PSUM") as ps_ob:

            QT = persist.tile([128, N_M, S], bf16)   # [p, m, s]: q^T row m*128+p
            KT = persist.tile([128, N_M, S], bf16)
            # V with a ones column appended per head: [p, st, h*65+c]
            Vx = persist.tile([128, N_KT, 8 * DKE], bf16)
            nc.vector.memset(Vx[:], 1.0)
            wo_sb = persist.tile([128, N_M, E], bf16)
            # sel2: row 0 selects denom A for partitions 0:64, row 1 -> 64:128
            sel2 = persist.tile([2, 128], f32)
            nc.sync.dma_start(sel2[:], sel[:])

            # ---------- phase 1: projections ----------
            with tc.tile_pool(name="xt", bufs=1) as xt_pool, \
                 tc.tile_pool(name="wchunk", bufs=10) as wch, \
                 tc.tile_pool(name="wvchunk", bufs=8) as wvch:
                xt = xt_pool.tile([128, N_EC, S], bf16)
                for ec in range(N_EC):
                    nc.sync.dma_start(xt[:, ec, :], xT[ec * 128:(ec + 1) * 128, :])
                for fo in range(N_M):
                    nc.sync.dma_start(wo_sb[:, fo, :], wo[fo * 128:(fo + 1) * 128, :])

                # Q^T and K^T: [dq, s] tiles; lhsT = w chunk, rhs = x^T chunk
                for dst, w_dram, wname in ((QT, wq, "q"), (KT, wk, "k")):
                    for m in range(N_M):
                        chunks = []
                        for ec in range(N_EC):
                            c = wch.tile([128, 128], bf16, name=f"w{wname}c")
                            nc.sync.dma_start(
                                c[:], w_dram[ec * 128:(ec + 1) * 128,
                                             m * 128:(m + 1) * 128])
                            chunks.append(c)
                        for sb in range(N_QB):
                            ps = ps_mm.tile([128, QB], f32, name="mm")
                            for ec in range(N_EC):
                                nc.tensor.matmul(
                                    ps[:], chunks[ec][:],
                                    xt[:, ec, sb * QB:(sb + 1) * QB],
                                    start=(ec == 0), stop=(ec == N_EC - 1))
                            nc.vector.tensor_copy(
                                dst[:, m, sb * QB:(sb + 1) * QB], ps[:])

                # V: [s, dv] tiles; lhsT = x^T chunk, rhs = w chunk
                vchunks = []
                for ec in range(N_EC):
                    c = wvch.tile([128, GH], bf16, name="wvc")
                    nc.sync.dma_start(c[:], wv[ec * 128:(ec + 1) * 128, :])
                    vchunks.append(c)
                for st in range(N_KT):
                    ps = ps_mm.tile([128, GH], f32, name="mm")
                    for ec in range(N_EC):
                        nc.tensor.matmul(
                            ps[:], xt[:, ec, st * 128:(st + 1) * 128],
                            vchunks[ec][:],
                            start=(ec == 0), stop=(ec == N_EC - 1))
                    # scatter into the ones-padded layout: [128, 8, 64] view
                    vdst = Vx[:, st, :].rearrange("p (h c) -> p h c", c=DKE)
                    nc.vector.tensor_copy(
                        vdst[:, :, 0:DK],
                        ps[:].rearrange("p (h c) -> p h c", c=DK))

            # ---------- phase 2: attention + out-proj, per query block ----------
            with tc.tile_pool(name="expp", bufs=4) as expp, \
                 tc.tile_pool(name="d2p", bufs=4) as d2p, \
                 tc.tile_pool(name="recipp", bufs=3) as recipp, \
                 tc.tile_pool(name="outTp", bufs=2) as outTp, \
                 tc.tile_pool(name="finp", bufs=4) as finp:
                for qb in range(N_QB):
                    qsl = slice(qb * QB, (qb + 1) * QB)
                    outT = outTp.tile([128, N_M, QB], bf16)  # [f, j, q]
                    for j in range(N_M):  # head pair j = heads (2j, 2j+1)
                        oTA = ps_oa.tile([DKE, QB], f32, name="ps_oTA")
                        oTB = ps_ob.tile([DKE, QB], f32, name="ps_oTB")
                        for kt in range(N_KT):
                            ksl = slice(kt * 128, (kt + 1) * 128)
                            # scores^T for both heads, row-packed, one psum tile
                            psS = ps_sc.tile([128, 2, QB], f32, name="psS")
                            nc.tensor.matmul(psS[:, 0, :], KT[0:64, j, ksl],
                                             QT[0:64, j, qsl])
                            nc.tensor.matmul(psS[:, 1, :], KT[64:128, j, ksl],
                                             QT[64:128, j, qsl])
                            # exp(score/8) for both heads in one ACT
                            eAB = expp.tile([128, 2, QB], bf16, name="eAB")
                            nc.scalar.activation(eAB[:], psS[:], Exp, scale=0.125)
                            # attn^T @ [V | 1] -> out rows 0:64, denom row 64
                            nc.tensor.matmul(
                                oTA[:, :],
                                Vx[:, kt, (2 * j) * DKE:(2 * j + 1) * DKE],
                                eAB[:, 0, :], start=(kt == 0),
                                stop=(kt == N_KT - 1))
                            nc.tensor.matmul(
                                oTB[:, :],
                                Vx[:, kt, (2 * j + 1) * DKE:(2 * j + 2) * DKE],
                                eAB[:, 1, :], start=(kt == 0),
                                stop=(kt == N_KT - 1))
                        # softmax denominators: batched reciprocal on 2 lanes
                        d2 = d2p.tile([2, QB], f32, name="d2")
                        nc.vector.tensor_copy(d2[0:1, :], oTA[DK:DKE, :])
                        # partition-1 writes need DMA (compute engines must
                        # start at aligned partitions); DMA can't read PSUM,
                        # so stage through SBUF
                        tmpB = d2p.tile([1, QB], f32, name="tmpB")
                        nc.vector.tensor_copy(tmpB[:], oTB[DK:DKE, :])
                        nc.sync.dma_start(d2[1:2, :], tmpB[:])
                        r2 = d2p.tile([2, QB], f32, name="r2")
                        nc.vector.reciprocal(r2[:], d2[:])
                        # broadcast: rows 0:64 <- 1/D_A, rows 64:128 <- 1/D_B
                        bcR = ps_mm.tile([128, QB], f32, name="mm")
                        nc.tensor.matmul(bcR[:], sel2[:], r2[:])
                        recipB = recipp.tile([128, QB], f32, name="recipB")
                        nc.vector.tensor_copy(recipB[:], bcR[:])
                        # evict attn output with the softmax divide fused
                        nc.vector.tensor_mul(outT[0:64, j, :], oTA[0:DK, :],
                                             recipB[0:64, :])
                        nc.vector.tensor_mul(outT[64:128, j, :], oTB[0:DK, :],
                                             recipB[64:128, :])
                    # out-proj for this query block: y[s, e] = outT^T @ wo
                    for st in range(QB // 128):
                        row0 = qb * QB + st * 128
                        for eb in range(E // QB):
                            ps = ps_mm.tile([128, QB], f32, name="mm")
                            for jj in range(N_M):
                                nc.tensor.matmul(
                                    ps[:], outT[:, jj, st * 128:(st + 1) * 128],
                                    wo_sb[:, jj, eb * QB:(eb + 1) * QB],
                                    start=(jj == 0), stop=(jj == N_M - 1))
                            fo = finp.tile([128, QB], f32, name="fo")
                            nc.vector.tensor_copy(fo[:], ps[:])
                            nc.sync.dma_start(
                                y[row0:row0 + 128, eb * QB:(eb + 1) * QB], fo[:])

    nc.finalize()
    return nc


def get_nc():
    global _NC
    if _NC is None:
        _NC = build_nc()
    return _NC


def _bf(a):
    return np.ascontiguousarray(a).astype(np.float16)


def make_in_maps(x, Wq, Wk, Wv, Wo):
    selmat = np.zeros((2, 128), np.float32)
    selmat[0, 0:64] = 1.0
    selmat[1, 64:128] = 1.0
    in_maps = []
    for b in range(B):
        xTb = _bf(np.asarray(x[b], np.float32).T)
        for g in range(2):
            sl = slice(g * GH, (g + 1) * GH)
            in_maps.append(dict(
                xT=xTb,
                wq=_bf(np.asarray(Wq, np.float32)[sl, :].T),
                wk=_bf(np.asarray(Wk, np.float32)[sl, :].T),
                wv=_bf(np.asarray(Wv, np.float32)[sl, :].T),
                wo=_bf(np.asarray(Wo, np.float32)[:, sl].T),
                sel=selmat,
            ))
    return in_maps


def combine(results):
    out = np.empty((B, S, E), np.float32)
    for b in range(B):
        out[b] = results[2 * b]["y"] + results[2 * b + 1]["y"]
    return out


def kernel(x, Wq, Wk, Wv, Wo):
    from concourse.bass_utils import run_bass_kernel_spmd
    res = run_bass_kernel_spmd(
        get_nc(), make_in_maps(x, Wq, Wk, Wv, Wo),
        core_ids=list(range(N_CORES)))
    return combine(res.results)


# revision 22
# speedup vs baseline: 1.2918x; 1.0526x over previous
"""Multi-head attention (b=4, s=2048, E=1024, 16 heads, d_k=64) on 8 trn2 cores.

Sharding: core = (batch b in 0..3, head-group g in 0..1); each core handles one
batch and 8 of the 16 heads (Megatron-style column-parallel QKV + row-parallel
out-proj). Each core returns its partial out-projection [2048, 1024]; the host
sums each batch's pair of partials in fp32 (the 2-way all-reduce, done during
unsharding).

Matmul operands are bf16 (fp32 PSUM accumulation). Scores are computed
transposed ([k, q] layout, two heads row-packed in the PE array, both written
into one 2-bank PSUM tile so a single ACT instruction applies exp with the
1/sqrt(64) scale fused). V carries an extra all-ones column so the attn^T @ V
matmul also produces the softmax denominator for free; a tiny batched DVE
reciprocal + ones-matmul broadcasts 1/denom across partitions, applied on the
PSUM eviction, which feeds the out-projection directly.
"""

import sys

if "/opt/trn_rl_repo" not in sys.path:
    sys.path.insert(0, "/opt/trn_rl_repo")

import numpy as np
import ml_dtypes

N_CORES = 8
B, S, E, H, DK = 4, 2048, 1024, 16, 64
GH = 512          # head-group width: 8 heads * 64
QB = 512          # query block (free dim of scores matmuls)
N_QB = S // QB    # 4
N_KT = S // 128   # 16 k-tiles
N_EC = E // 128   # 8 contraction chunks for projections
N_M = GH // 128   # 4 dq tiles / head pairs
DKE = DK + 1      # V head width incl the ones column

_NC = None


def build_nc():
    import concourse.tile as tile
    from concourse import bacc, mybir

    f32 = mybir.dt.float32
    bf16 = mybir.dt.float16  # fp16: same PE rate, 4x better mantissa than bf16
    Exp = mybir.ActivationFunctionType.Exp
    Ln = mybir.ActivationFunctionType.Ln

    nc = bacc.Bacc("TRN2", target_bir_lowering=False, debug=False,
                   num_devices=N_CORES)

    xT = nc.dram_tensor("xT", [E, S], bf16, kind="ExternalInput")
    wq = nc.dram_tensor("wq", [E, GH], bf16, kind="ExternalInput")
    wk = nc.dram_tensor("wk", [E, GH], bf16, kind="ExternalInput")
    wv = nc.dram_tensor("wv", [E, GH], bf16, kind="ExternalInput")
    wo = nc.dram_tensor("wo", [GH, E], bf16, kind="ExternalInput")
    sel = nc.dram_tensor("sel", [2, 128], f32, kind="ExternalInput")
    y = nc.dram_tensor("y", [S, E], f32, kind="ExternalOutput")

    with tile.TileContext(nc) as tc:
        with tc.tile_pool(name="persist", bufs=1) as persist, \
             tc.tile_pool(name="ps_mm", bufs=2, space="PSUM") as ps_mm, \
             tc.tile_pool(name="ps_sc", bufs=2, space="PSUM") as ps_sc, \
             tc.tile_pool(name="ps_oa", bufs=1, space="PSUM") as ps_oa, \
             tc.tile_pool(name="ps_ob", bufs=1, space="PSUM") as ps_ob:

            QT = persist.tile([128, N_M, S], bf16)   # [p, m, s]: q^T row m*128+p
            KT = persist.tile([128, N_M, S], bf16)
            # V with a ones column appended per head: [p, st, h*65+c]
            Vx = persist.tile([128, N_KT, 8 * DKE], bf16)
            nc.vector.memset(Vx[:], 1.0)
            wo_sb = persist.tile([128, N_M, E], bf16)
            # selA broadcasts denom A to partitions 0:64, selB to 64:128
            selA = persist.tile([1, 128], f32)
            nc.sync.dma_start(selA[:], sel[0:1, :])
            selB = persist.tile([1, 128], f32)
            nc.sync.dma_start(selB[:], sel[1:2, :])

            # ---------- phase 1: projections ----------
            with tc.tile_pool(name="xt", bufs=1) as xt_pool, \
                 tc.tile_pool(name="wchunk", bufs=10) as wch, \
                 tc.tile_pool(name="wvchunk", bufs=8) as wvch:
                xt = xt_pool.tile([128, N_EC, S], bf16)
                for ec in range(N_EC):
                    nc.sync.dma_start(xt[:, ec, :], xT[ec * 128:(ec + 1) * 128, :])
                for fo in range(N_M):
                    nc.sync.dma_start(wo_sb[:, fo, :], wo[fo * 128:(fo + 1) * 128, :])

                # Q^T and K^T: [dq, s] tiles; lhsT = w chunk, rhs = x^T chunk
                for dst, w_dram, wname in ((QT, wq, "q"), (KT, wk, "k")):
                    for m in range(N_M):
                        chunks = []
                        for ec in range(N_EC):
                            c = wch.tile([128, 128], bf16, name=f"w{wname}c")
                            nc.sync.dma_start(
                                c[:], w_dram[ec * 128:(ec + 1) * 128,
                                             m * 128:(m + 1) * 128])
                            chunks.append(c)
                        for sb in range(N_QB):
                            ps = ps_mm.tile([128, QB], f32, name="mm")
                            for ec in range(N_EC):
                                nc.tensor.matmul(
                                    ps[:], chunks[ec][:],
                                    xt[:, ec, sb * QB:(sb + 1) * QB],
                                    start=(ec == 0), stop=(ec == N_EC - 1))
                            nc.vector.tensor_copy(
                                dst[:, m, sb * QB:(sb + 1) * QB], ps[:])

                # V: [s, dv] tiles; lhsT = x^T chunk, rhs = w chunk
                vchunks = []
                for ec in range(N_EC):
                    c = wvch.tile([128, GH], bf16, name="wvc")
                    nc.sync.dma_start(c[:], wv[ec * 128:(ec + 1) * 128, :])
                    vchunks.append(c)
                for st in range(N_KT):
                    ps = ps_mm.tile([128, GH], f32, name="mm")
                    for ec in range(N_EC):
                        nc.tensor.matmul(
                            ps[:], xt[:, ec, st * 128:(st + 1) * 128],
                            vchunks[ec][:],
                            start=(ec == 0), stop=(ec == N_EC - 1))
                    # scatter into the ones-padded layout: [128, 8, 64] view
                    vdst = Vx[:, st, :].rearrange("p (h c) -> p h c", c=DKE)
                    nc.vector.tensor_copy(
                        vdst[:, :, 0:DK],
                        ps[:].rearrange("p (h c) -> p h c", c=DK))

            # ---------- phase 2: attention + out-proj, per query block ----------
            with tc.tile_pool(name="expp", bufs=6) as expp, \
                 tc.tile_pool(name="d2p", bufs=4) as d2p, \
                 tc.tile_pool(name="recipp", bufs=3) as recipp, \
                 tc.tile_pool(name="outTp", bufs=2) as outTp, \
                 tc.tile_pool(name="finp", bufs=4) as finp:
                for qb in range(N_QB):
                    qsl = slice(qb * QB, (qb + 1) * QB)
                    outT = outTp.tile([128, N_M, QB], bf16)  # [f, j, q]
                    for j in range(N_M):  # head pair j = heads (2j, 2j+1)
                        oTA = ps_oa.tile([DKE, QB], f32, name="ps_oTA")
                        oTB = ps_ob.tile([DKE, QB], f32, name="ps_oTB")
                        for kt in range(N_KT):
                            ksl = slice(kt * 128, (kt + 1) * 128)
                            # scores^T for both heads, row-packed, one psum tile
                            psS = ps_sc.tile([128, 2, QB], f32, name="psS")
                            nc.tensor.matmul(psS[:, 0, :], KT[0:64, j, ksl],
                                             QT[0:64, j, qsl])
                            nc.tensor.matmul(psS[:, 1, :], KT[64:128, j, ksl],
                                             QT[64:128, j, qsl])
                            # exp(score/8) for both heads in one ACT
                            eAB = expp.tile([128, 2, QB], bf16, name="eAB")
                            nc.scalar.activation(eAB[:], psS[:], Exp, scale=0.125)
                            # attn^T @ [V | 1] -> out rows 0:64, denom row 64
                            nc.tensor.matmul(
                                oTA[:, :],
                                Vx[:, kt, (2 * j) * DKE:(2 * j + 1) * DKE],
                                eAB[:, 0, :], start=(kt == 0),
                                stop=(kt == N_KT - 1))
                            nc.tensor.matmul(
                                oTB[:, :],
                                Vx[:, kt, (2 * j + 1) * DKE:(2 * j + 2) * DKE],
                                eAB[:, 1, :], start=(kt == 0),
                                stop=(kt == N_KT - 1))
                        # softmax 1/denom via ln -> broadcast -> exp(-x)
                        # (Exp and Ln share an ACT table set; DVE reciprocal
                        # is ~3.3us/call)
                        lnA = d2p.tile([1, QB], f32, name="lnA")
                        nc.scalar.activation(lnA[:], oTA[DK:DKE, :], Ln)
                        lnB = d2p.tile([1, QB], f32, name="lnB")
                        nc.scalar.activation(lnB[:], oTB[DK:DKE, :], Ln)
                        bcR = ps_mm.tile([128, QB], f32, name="mm")
                        nc.tensor.matmul(bcR[:], selA[:], lnA[:],
                                         start=True, stop=False)
                        nc.tensor.matmul(bcR[:], selB[:], lnB[:],
                                         start=False, stop=True)
                        recipB = recipp.tile([128, QB], f32, name="recipB")
                        nc.scalar.activation(recipB[:], bcR[:], Exp, scale=-1.0)
                        # evict attn output with the softmax divide fused
                        nc.vector.tensor_mul(outT[0:64, j, :], oTA[0:DK, :],
                                             recipB[0:64, :])
                        nc.vector.tensor_mul(outT[64:128, j, :], oTB[0:DK, :],
                                             recipB[64:128, :])
                    # out-proj for this query block: y[s, e] = outT^T @ wo
                    # (eb inner so each outT chunk is loaded as weights once)
                    for st in range(QB // 128):
                        row0 = qb * QB + st * 128
                        pss = [ps_mm.tile([128, QB], f32, name="mm")
                               for _ in range(E // QB)]
                        for jj in range(N_M):
                            for eb in range(E // QB):
                                nc.tensor.matmul(
                                    pss[eb][:],
                                    outT[:, jj, st * 128:(st + 1) * 128],
                                    wo_sb[:, jj, eb * QB:(eb + 1) * QB],
                                    start=(jj == 0), stop=(jj == N_M - 1))
                        for eb in range(E // QB):
                            fo = finp.tile([128, QB], f32, name="fo")
                            nc.vector.tensor_copy(fo[:], pss[eb][:])
                            nc.sync.dma_start(
                                y[row0:row0 + 128, eb * QB:(eb + 1) * QB], fo[:])

    nc.finalize()
    return nc


def get_nc():
    global _NC
    if _NC is None:
        _NC = build_nc()
    return _NC


def _bf(a):
    return np.ascontiguousarray(a).astype(np.float16)


def make_in_maps(x, Wq, Wk, Wv, Wo):
    selmat = np.zeros((2, 128), np.float32)
    selmat[0, 0:64] = 1.0
    selmat[1, 64:128] = 1.0
    in_maps = []
    for b in range(B):
        xTb = _bf(np.asarray(x[b], np.float32).T)
        for g in range(2):
            sl = slice(g * GH, (g + 1) * GH)
            in_maps.append(dict(
                xT=xTb,
                wq=_bf(np.asarray(Wq, np.float32)[sl, :].T),
                wk=_bf(np.asarray(Wk, np.float32)[sl, :].T),
                wv=_bf(np.asarray(Wv, np.float32)[sl, :].T),
                wo=_bf(np.asarray(Wo, np.float32)[:, sl].T),
                sel=selmat,
            ))
    return in_maps


def combine(results):
    out = np.empty((B, S, E), np.float32)
    for b in range(B):
        out[b] = results[2 * b]["y"] + results[2 * b + 1]["y"]
    return out


def kernel(x, Wq, Wk, Wv, Wo):
    from concourse.bass_utils import run_bass_kernel_spmd
    res = run_bass_kernel_spmd(
        get_nc(), make_in_maps(x, Wq, Wk, Wv, Wo),
        core_ids=list(range(N_CORES)))
    return combine(res.results)


# revision 28
# speedup vs baseline: 1.4920x; 1.1550x over previous
"""Multi-head attention (b=4, s=2048, E=1024, 16 heads, d_k=64) on 8 trn2 cores.

Sharding: core = (batch b in 0..3, head-group g in 0..1); each core handles one
batch and 8 of the 16 heads (Megatron-style column-parallel QKV + row-parallel
out-proj). Each core returns its partial out-projection [2048, 1024]; the host
sums each batch's pair of partials in fp32 (the 2-way all-reduce, done during
unsharding).

Matmul operands are bf16 (fp32 PSUM accumulation). Scores are computed
transposed ([k, q] layout, two heads row-packed in the PE array, both written
into one 2-bank PSUM tile so a single ACT instruction applies exp with the
1/sqrt(64) scale fused). V carries an extra all-ones column so the attn^T @ V
matmul also produces the softmax denominator for free; a tiny batched DVE
reciprocal + ones-matmul broadcasts 1/denom across partitions, applied on the
PSUM eviction, which feeds the out-projection directly.
"""

import sys

if "/opt/trn_rl_repo" not in sys.path:
    sys.path.insert(0, "/opt/trn_rl_repo")

import numpy as np
import ml_dtypes

N_CORES = 8
B, S, E, H, DK = 4, 2048, 1024, 16, 64
GH = 512          # head-group width: 8 heads * 64
QB = 512          # query block (free dim of scores matmuls)
N_QB = S // QB    # 4
N_KT = S // 128   # 16 k-tiles
N_EC = E // 128   # 8 contraction chunks for projections
N_M = GH // 128   # 4 dq tiles / head pairs
DKE = DK + 1      # V head width incl the ones column

_NC = None


def build_nc():
    import concourse.tile as tile
    from concourse import bacc, mybir

    f32 = mybir.dt.float32
    bf16 = mybir.dt.float16  # fp16: same PE rate, 4x better mantissa than bf16
    Exp = mybir.ActivationFunctionType.Exp
    Copy = mybir.ActivationFunctionType.Copy

    nc = bacc.Bacc("TRN2", target_bir_lowering=False, debug=False,
                   num_devices=N_CORES)

    xT = nc.dram_tensor("xT", [E, S], bf16, kind="ExternalInput")
    wq = nc.dram_tensor("wq", [E, GH], bf16, kind="ExternalInput")
    wk = nc.dram_tensor("wk", [E, GH], bf16, kind="ExternalInput")
    wv = nc.dram_tensor("wv", [E, GH], bf16, kind="ExternalInput")
    wo = nc.dram_tensor("wo", [GH, E], bf16, kind="ExternalInput")
    sel = nc.dram_tensor("sel", [2, 128], bf16, kind="ExternalInput")
    y = nc.dram_tensor("y", [S, E], f32, kind="ExternalOutput")

    with tile.TileContext(nc) as tc:
        with tc.tile_pool(name="persist", bufs=1) as persist, \
             tc.tile_pool(name="ps_mm", bufs=2, space="PSUM") as ps_mm, \
             tc.tile_pool(name="ps_sc", bufs=2, space="PSUM") as ps_sc, \
             tc.tile_pool(name="ps_oa", bufs=1, space="PSUM") as ps_oa, \
             tc.tile_pool(name="ps_ob", bufs=1, space="PSUM") as ps_ob:

            QT = persist.tile([128, N_M, S], bf16)   # [p, m, s]: q^T row m*128+p
            KT = persist.tile([128, N_M, S], bf16)
            # V with a ones column appended per head: [p, st, h*65+c]
            Vx = persist.tile([128, N_KT, 8 * DKE], bf16)
            nc.vector.memset(Vx[:], 1.0)
            wo_sb = persist.tile([128, N_M, E], bf16)
            # selA broadcasts denom A to partitions 0:64, selB to 64:128
            selA = persist.tile([1, 128], bf16)
            nc.sync.dma_start(selA[:], sel[0:1, :])
            selB = persist.tile([1, 128], bf16)
            nc.sync.dma_start(selB[:], sel[1:2, :])

            # ---------- phase 1: projections ----------
            with tc.tile_pool(name="xt", bufs=1) as xt_pool, \
                 tc.tile_pool(name="wchunk", bufs=10) as wch, \
                 tc.tile_pool(name="wvchunk", bufs=8) as wvch:
                xt = xt_pool.tile([128, N_EC, S], bf16)
                for ec in range(N_EC):
                    nc.sync.dma_start(xt[:, ec, :], xT[ec * 128:(ec + 1) * 128, :])
                for fo in range(N_M):
                    nc.sync.dma_start(wo_sb[:, fo, :], wo[fo * 128:(fo + 1) * 128, :])

                # Q^T and K^T: [dq, s] tiles; lhsT = w chunk, rhs = x^T chunk
                for dst, w_dram, wname in ((QT, wq, "q"), (KT, wk, "k")):
                    for m in range(N_M):
                        chunks = []
                        for ec in range(N_EC):
                            c = wch.tile([128, 128], bf16, name=f"w{wname}c")
                            nc.sync.dma_start(
                                c[:], w_dram[ec * 128:(ec + 1) * 128,
                                             m * 128:(m + 1) * 128])
                            chunks.append(c)
                        for sb in range(N_QB):
                            ps = ps_mm.tile([128, QB], f32, name="mm")
                            for ec in range(N_EC):
                                nc.tensor.matmul(
                                    ps[:], chunks[ec][:],
                                    xt[:, ec, sb * QB:(sb + 1) * QB],
                                    start=(ec == 0), stop=(ec == N_EC - 1))
                            nc.vector.tensor_copy(
                                dst[:, m, sb * QB:(sb + 1) * QB], ps[:])

                # V: [s, dv] tiles; lhsT = x^T chunk, rhs = w chunk
                vchunks = []
                for ec in range(N_EC):
                    c = wvch.tile([128, GH], bf16, name="wvc")
                    nc.sync.dma_start(c[:], wv[ec * 128:(ec + 1) * 128, :])
                    vchunks.append(c)
                for st in range(N_KT):
                    ps = ps_mm.tile([128, GH], f32, name="mm")
                    for ec in range(N_EC):
                        nc.tensor.matmul(
                            ps[:], xt[:, ec, st * 128:(st + 1) * 128],
                            vchunks[ec][:],
                            start=(ec == 0), stop=(ec == N_EC - 1))
                    # scatter into the ones-padded layout: [128, 8, 64] view
                    vdst = Vx[:, st, :].rearrange("p (h c) -> p h c", c=DKE)
                    nc.vector.tensor_copy(
                        vdst[:, :, 0:DK],
                        ps[:].rearrange("p (h c) -> p h c", c=DK))

            # ---------- phase 2: attention + out-proj, per query block ----------
            with tc.tile_pool(name="expp", bufs=6) as expp, \
                 tc.tile_pool(name="d2p", bufs=4) as d2p, \
                 tc.tile_pool(name="recipp", bufs=3) as recipp, \
                 tc.tile_pool(name="outTp", bufs=2) as outTp, \
                 tc.tile_pool(name="finp", bufs=4) as finp:
                for qb in range(N_QB):
                    qsl = slice(qb * QB, (qb + 1) * QB)
                    outT = outTp.tile([128, N_M, QB], bf16)  # [f, j, q]
                    for j in range(N_M):  # head pair j = heads (2j, 2j+1)
                        oTA = ps_oa.tile([DKE, QB], f32, name="ps_oTA")
                        oTB = ps_ob.tile([DKE, QB], f32, name="ps_oTB")
                        for kt in range(N_KT):
                            ksl = slice(kt * 128, (kt + 1) * 128)
                            # scores^T for both heads, row-packed, one psum tile
                            psS = ps_sc.tile([128, 2, QB], f32, name="psS")
                            nc.tensor.matmul(psS[:, 0, :], KT[0:64, j, ksl],
                                             QT[0:64, j, qsl])
                            nc.tensor.matmul(psS[:, 1, :], KT[64:128, j, ksl],
                                             QT[64:128, j, qsl])
                            # exp(score/8) for both heads in one ACT
                            eAB = expp.tile([128, 2, QB], bf16, name="eAB")
                            nc.scalar.activation(eAB[:], psS[:], Exp, scale=0.125)
                            # attn^T @ [V | 1] -> out rows 0:64, denom row 64
                            nc.tensor.matmul(
                                oTA[:, :],
                                Vx[:, kt, (2 * j) * DKE:(2 * j + 1) * DKE],
                                eAB[:, 0, :], start=(kt == 0),
                                stop=(kt == N_KT - 1))
                            nc.tensor.matmul(
                                oTB[:, :],
                                Vx[:, kt, (2 * j + 1) * DKE:(2 * j + 2) * DKE],
                                eAB[:, 1, :], start=(kt == 0),
                                stop=(kt == N_KT - 1))
                        # softmax 1/denom: matmul-broadcast the two denom
                        # rows (fp16) across partitions, then a single
                        # fast-approx DVE reciprocal off PSUM (~18-bit,
                        # ~0.7us) that doubles as the eviction.
                        dA = d2p.tile([1, QB], bf16, name="dA")
                        nc.vector.tensor_copy(dA[:], oTA[DK:DKE, :])
                        dB = d2p.tile([1, QB], bf16, name="dB")
                        nc.vector.tensor_copy(dB[:], oTB[DK:DKE, :])
                        bcD = ps_mm.tile([128, QB], f32, name="mm")
                        nc.tensor.matmul(bcD[:], selA[:], dA[:],
                                         start=True, stop=False)
                        nc.tensor.matmul(bcD[:], selB[:], dB[:],
                                         start=False, stop=True)
                        recipB = recipp.tile([128, QB], f32, name="recipB")
                        nc.vector.reciprocal_approx_fast(recipB[:], bcD[:])
                        # evict attn output with the softmax divide fused
                        nc.vector.tensor_mul(outT[0:64, j, :], oTA[0:DK, :],
                                             recipB[0:64, :])
                        nc.vector.tensor_mul(outT[64:128, j, :], oTB[0:DK, :],
                                             recipB[64:128, :])
                    # out-proj for this query block: y[s, e] = outT^T @ wo
                    # (eb inner so each outT chunk is loaded as weights once)
                    for st in range(QB // 128):
                        row0 = qb * QB + st * 128
                        pss = [ps_mm.tile([128, QB], f32, name="mm")
                               for _ in range(E // QB)]
                        for jj in range(N_M):
                            for eb in range(E // QB):
                                nc.tensor.matmul(
                                    pss[eb][:],
                                    outT[:, jj, st * 128:(st + 1) * 128],
                                    wo_sb[:, jj, eb * QB:(eb + 1) * QB],
                                    start=(jj == 0), stop=(jj == N_M - 1))
                        for eb in range(E // QB):
                            fo = finp.tile([128, QB], f32, name="fo")
                            nc.vector.tensor_copy(fo[:], pss[eb][:])
                            nc.sync.dma_start(
                                y[row0:row0 + 128, eb * QB:(eb + 1) * QB], fo[:])

    nc.finalize()
    return nc


def get_nc():
    global _NC
    if _NC is None:
        _NC = build_nc()
    return _NC


def _bf(a):
    return np.ascontiguousarray(a).astype(np.float16)


def make_in_maps(x, Wq, Wk, Wv, Wo):
    selmat = np.zeros((2, 128), np.float16)
    selmat[0, 0:64] = 1.0
    selmat[1, 64:128] = 1.0
    in_maps = []
    for b in range(B):
        xTb = _bf(np.asarray(x[b], np.float32).T)
        for g in range(2):
            sl = slice(g * GH, (g + 1) * GH)
            in_maps.append(dict(
                xT=xTb,
                wq=_bf(np.asarray(Wq, np.float32)[sl, :].T),
                wk=_bf(np.asarray(Wk, np.float32)[sl, :].T),
                wv=_bf(np.asarray(Wv, np.float32)[sl, :].T),
                wo=_bf(np.asarray(Wo, np.float32)[:, sl].T),
                sel=selmat,
            ))
    return in_maps


def combine(results):
    out = np.empty((B, S, E), np.float32)
    for b in range(B):
        out[b] = results[2 * b]["y"] + results[2 * b + 1]["y"]
    return out


def kernel(x, Wq, Wk, Wv, Wo):
    from concourse.bass_utils import run_bass_kernel_spmd
    res = run_bass_kernel_spmd(
        get_nc(), make_in_maps(x, Wq, Wk, Wv, Wo),
        core_ids=list(range(N_CORES)))
    return combine(res.results)
